# revision 54
# baseline (speedup 1.0000x reference)
"""Tensor-parallel LLaMA attention block on 8 TRN2 NeuronCores.

Sharding (Megatron over heads): core c owns heads [4c, 4c+4).
  - QKV projections column-sharded (bf16 matmuls, weights host-permuted so
    rotary even/odd pairs land de-interleaved: scores are permutation
    invariant along head_dim).
  - Rotary applied on DVE in [d, tok] layout for Q/K.
  - Attention per (b, head) in f32r: scores computed transposed
    ST[kt, qt] = K_dT.T @ Q_dT, causal tiles only, mask folded pre-exp,
    softmax denominator via ones-vector matmul, normalization applied
    post-PV with an outer-product broadcast tile.
  - attnT (d-major) AllGathered in bf16 across cores, then the output
    projection is output-dim sharded: each core computes
    outT_c = wo_c @ attn_full for all tokens (bf16).
Host side shards/preps inputs, assembles the full (out, (cache_k, cache_v)).
"""

import sys

for _p in ("/opt/trn_rl_repo", "/opt/pypackages"):
    if _p not in sys.path:
        sys.path.insert(0, _p)

import math

import numpy as np
import ml_dtypes

from concourse import mybir, tile, bacc
from concourse.bass_utils import run_bass_kernel_spmd

F32 = mybir.dt.float32
F32R = mybir.dt.float32r
BF16 = mybir.dt.bfloat16
BF = ml_dtypes.bfloat16

N_CORES = 8


class Cfg:
    def __init__(self, B=2, S=2048, D=4096, H=32, n_cores=N_CORES, hl_override=None):
        self.B, self.S, self.D, self.H = B, S, D, H
        self.HD = 128                      # head dim (fixed: partition width)
        self.hl_override = hl_override
        self.HL = hl_override if hl_override else H // n_cores
        self.DL = self.HL * self.HD        # local width (q/k/v out dims)
        self.T = B * S                     # total tokens
        self.DT = D // 128                 # D tiles (contraction)
        self.TT = self.T // 128            # token tiles
        self.TC = self.T // 512            # 512-token chunks
        self.SC = S // 512                 # chunks per batch row
        self.KT = S // 128                 # key tiles per batch row
        self.n_cores = n_cores
        if not hl_override:
            assert self.HL * n_cores == H
        assert D % 128 == 0 and S % 512 == 0


def build(cfg: Cfg, phases="ABCD", es_bufs=3, pst_bufs=3, pden_bufs=2):
    nc = bacc.Bacc(
        "TRN2", target_bir_lowering=False, debug=False, num_devices=cfg.n_cores
    )
    B, S, D, T = cfg.B, cfg.S, cfg.D, cfg.T
    DT, TT, TC, SC, KT = cfg.DT, cfg.TT, cfg.TC, cfg.SC, cfg.KT
    HL, DL = cfg.HL, cfg.DL
    SCALE = 1.0 / math.sqrt(cfg.HD)

    # ---- parameters (per-core shards; host preps layouts) ----
    # xt: x^T tiled [128 d-in-tile, DT d-tile, T tok] bf16
    xt_ext = nc.declare_dram_parameter("xt", [128, DT, T], BF16, isOutput=False)
    # wqk: [128, DT, 2*DL] bf16 (cols: q local dims | k local dims, rotary-permuted)
    wqk_ext = nc.declare_dram_parameter("wqk", [128, DT, 2 * DL], BF16, isOutput=False)
    wv_ext = nc.declare_dram_parameter("wv", [128, DT, DL], BF16, isOutput=False)
    # wo: wo_c^T tiled [128, DT, DL] bf16 (od-sharded rows of wo)
    wo_ext = nc.declare_dram_parameter("wo", [128, DT, DL], BF16, isOutput=False)
    # rotary, transposed: [64, S] f32
    cos_ext = nc.declare_dram_parameter("cosT", [64, S], F32, isOutput=False)
    sin_ext = nc.declare_dram_parameter("sinT", [64, S], F32, isOutput=False)
    # transposed+prescaled diagonal mask tile [128, 128] f32
    mt_ext = nc.declare_dram_parameter("maskt", [128, 128], F32, isOutput=False)
    ident_ext = nc.declare_dram_parameter("ident", [128, 128], F32R, isOutput=False)
    ones_ext = nc.declare_dram_parameter("ones", [128, 128], F32R, isOutput=False)

    outt_ext = nc.declare_dram_parameter("outt", [DL, T], F32, isOutput=True)
    ck_ext = nc.declare_dram_parameter("ck", [B, S, HL, 128], F32, isOutput=True)
    cv_ext = nc.declare_dram_parameter("cv", [B, S, HL, 128], F32, isOutput=True)

    with tile.TileContext(nc) as tc:
        with tc.tile_pool(name="dram", bufs=1, space="DRAM") as dram:
            q_spill = dram.tile([DL, T], F32R)     # Q' d-major
            k_spill = dram.tile([DL, T], F32R)     # K' d-major
            v_spill = dram.tile([T, DL], F32R)     # V natural
            ag_ins = [dram.tile([DL, S], BF16, name=f"ag_in{b}") for b in range(B)]
            ag_outs = [
                dram.tile(
                    [DL * cfg.n_cores, S], BF16, addr_space="Shared", name=f"ag_out{b}"
                )
                for b in range(B)
            ]

            # ================= phase A: Q/K projections + rotary =============
            with (
                tc.tile_pool(name="qk_sb", bufs=1) as sb,
                tc.tile_pool(name="qk_sb2", bufs=2) as sb2,
                tc.tile_pool(name="qk_ps", bufs=3, space="PSUM") as ps,
            ):
                wqk = sb.tile([128, DT, 2 * DL], BF16)
                nc.sync.dma_start(wqk[:], wqk_ext[:])
                if "M" in phases:
                    wv_m = sb.tile([128, DT, DL], BF16)
                    nc.sync.dma_start(wv_m[:], wv_ext[:])
                cosT = sb.tile([64, S], F32)
                sinT = sb.tile([64, S], F32)
                nc.sync.dma_start(cosT[:], cos_ext[:])
                nc.sync.dma_start(sinT[:], sin_ext[:])
                ident = sb.tile([128, 128], F32R)
                nc.sync.dma_start(ident[:], ident_ext[:])

                for n in range(TC if "A" in phases else 0):  # 512-token chunks
                    t0 = n * 512
                    s0 = t0 % S              # position within batch row
                    b = t0 // S
                    xt = sb2.tile([128, DT, 512], BF16, tag="xt")
                    nc.sync.dma_start(xt[:], xt_ext[:, :, t0 : t0 + 512])
                    for dq in range(2 * HL):  # q heads then k heads
                        is_k = dq >= HL
                        hl = dq - HL if is_k else dq
                        pqk = ps.tile([128, 512], F32, tag="pqk")
                        for a in range(DT):
                            nc.tensor.matmul(
                                pqk[:],
                                wqk[:, a, dq * 128 : (dq + 1) * 128],
                                xt[:, a, :],
                                start=(a == 0),
                                stop=(a == DT - 1),
                            )
                        # rotary: rows 0:64 = even pairs, 64:128 = odd pairs
                        st = sb2.tile([128, 512], F32R, tag="rot")
                        if "r" in phases:
                            nc.vector.tensor_copy(st[:], pqk[:])
                        else:
                            c_sl = cosT[:, s0 : s0 + 512]
                            s_sl = sinT[:, s0 : s0 + 512]
                            # one temp tile: halves align base partitions with st
                            tmp = sb2.tile([128, 512], F32, tag="rtmp")
                            # e' = e*c - o*s ; o' = e*s + o*c
                            nc.vector.tensor_mul(tmp[0:64, :], pqk[64:128, :], s_sl)
                            nc.vector.tensor_mul(st[0:64, :], pqk[0:64, :], c_sl)
                            nc.vector.tensor_sub(st[0:64, :], st[0:64, :], tmp[0:64, :])
                            nc.vector.tensor_mul(tmp[64:128, :], pqk[0:64, :], s_sl)
                            nc.vector.tensor_mul(st[64:128, :], pqk[64:128, :], c_sl)
                            nc.vector.tensor_add(
                                st[64:128, :], st[64:128, :], tmp[64:128, :]
                            )
                        spill = k_spill if is_k else q_spill
                        nc.sync.dma_start(
                            spill[hl * 128 : (hl + 1) * 128, t0 : t0 + 512], st[:]
                        )
                        if is_k and "x" not in phases:
                            # cache_k needs [tok, d] re-interleaved
                            kc = sb2.tile([128, 512], F32, tag="kc")
                            for tt in range(4):  # 128-tok tiles in chunk
                                ptr = ps.tile([128, 128], F32R, tag="ptr")
                                nc.tensor.transpose(
                                    ptr[:], st[:, tt * 128 : (tt + 1) * 128], ident[:]
                                )
                                ii = tt * 128
                                kc_e = kc[:, ii : ii + 128].rearrange(
                                    "p (x two) -> p two x", two=2
                                )
                                nc.vector.tensor_copy(kc_e[:, 0, :], ptr[:, 0:64])
                                nc.vector.tensor_copy(kc_e[:, 1, :], ptr[:, 64:128])
                            for tt in range(4):
                                nc.sync.dma_start(
                                    ck_ext[
                                        b, s0 + tt * 128 : s0 + (tt + 1) * 128, hl, :
                                    ],
                                    kc[:, tt * 128 : (tt + 1) * 128],
                                )
                    if "M" in phases:
                        for tt in range(4):
                            pv = ps.tile([128, DL], F32, tag="pv", bufs=2)
                            for a in range(DT):
                                nc.tensor.matmul(
                                    pv[:],
                                    xt[:, a, tt * 128 : (tt + 1) * 128],
                                    wv_m[:, a, :],
                                    start=(a == 0),
                                    stop=(a == DT - 1),
                                )
                            vs = sb2.tile([128, DL], F32R, tag="vs")
                            nc.vector.tensor_copy(vs[:], pv[:])
                            tq = t0 + tt * 128
                            nc.sync.dma_start(v_spill[tq : tq + 128, :], vs[:])
                            nc.sync.dma_start(
                                cv_ext[b, s0 + tt * 128 : s0 + (tt + 1) * 128, :, :],
                                vs[:].bitcast(F32).rearrange(
                                    "p (h d) -> p h d", d=128
                                ),
                            )

            # ================= phase B: V projection ========================
            with (
                tc.tile_pool(name="v_sb", bufs=1) as sb,
                tc.tile_pool(name="v_sb2", bufs=2) as sb2,
                tc.tile_pool(name="v_ps", bufs=3, space="PSUM") as ps,
            ):
                if "B" in phases:
                    wv = sb.tile([128, DT, DL], BF16)
                    nc.sync.dma_start(wv[:], wv_ext[:])
                for n in range(TC if "B" in phases else 0):
                    t0 = n * 512
                    b = t0 // S
                    s0 = t0 % S
                    xt = sb2.tile([128, DT, 512], BF16, tag="xt")
                    nc.sync.dma_start(xt[:], xt_ext[:, :, t0 : t0 + 512])
                    for tt in range(4):
                        pv = ps.tile([128, DL], F32, tag="pv")
                        for a in range(DT):
                            nc.tensor.matmul(
                                pv[:],
                                xt[:, a, tt * 128 : (tt + 1) * 128],
                                wv[:, a, :],
                                start=(a == 0),
                                stop=(a == DT - 1),
                            )
                        vs = sb2.tile([128, DL], F32R, tag="vs")
                        nc.vector.tensor_copy(vs[:], pv[:])
                        tq = t0 + tt * 128
                        nc.sync.dma_start(v_spill[tq : tq + 128, :], vs[:])
                        nc.sync.dma_start(
                            cv_ext[b, s0 + tt * 128 : s0 + (tt + 1) * 128, :, :],
                            vs[:].bitcast(F32).rearrange("p (h d) -> p h d", d=128),
                        )

            # ================= phase C: attention per (b, head) =============
            with (
                tc.tile_pool(name="at_sb", bufs=1) as sb,
                tc.tile_pool(name="at_sb2", bufs=2) as sb2,
                tc.tile_pool(name="at_es", bufs=3) as esb,
                tc.tile_pool(name="at_ps", bufs=2, space="PSUM") as ps,
                tc.tile_pool(name="at_ps3", bufs=3, space="PSUM") as ps3,
            ):
                maskt = sb.tile([128, 128], F32)
                nc.sync.dma_start(maskt[:], mt_ext[:])
                ones_t = sb.tile([128, 128], F32R)
                nc.sync.dma_start(ones_t[:], ones_ext[:])
                ones128 = ones_t[:, 0:1]   # [128, 1] lhsT for den sums
                ones1 = ones_t[0:1, :]     # [1, 128] lhsT for broadcast

                for b in range(B if "C" in phases else 0):
                    for hl in range(HL):
                        qd = sb2.tile([128, S], F32R, tag="qd")
                        kd = sb2.tile([128, S], F32R, tag="kd")
                        vv = sb2.tile([128, KT, 128], F32R, tag="vv")
                        nc.sync.dma_start(
                            qd[:], q_spill[hl * 128 : (hl + 1) * 128, b * S : (b + 1) * S]
                        )
                        nc.sync.dma_start(
                            kd[:], k_spill[hl * 128 : (hl + 1) * 128, b * S : (b + 1) * S]
                        )
                        for j in range(KT):
                            nc.sync.dma_start(
                                vv[:, j, :],
                                v_spill[
                                    b * S + j * 128 : b * S + (j + 1) * 128,
                                    hl * 128 : (hl + 1) * 128,
                                ],
                            )
                        for C in range(SC):
                            nj = 4 * C + 4
                            po = ps.tile([128, 512], F32, tag="po")
                            pden = ps.tile([1, 512], F32, tag="pden", bufs=pden_bufs)
                            for j in range(nj):
                                r = j - 4 * C
                                c0 = max(0, r) * 128  # first valid qt col
                                pst = ps3.tile([128, 512], F32, tag="pst", bufs=pst_bufs)
                                nc.tensor.matmul(
                                    pst[:, c0:512],
                                    kd[:, j * 128 : (j + 1) * 128],
                                    qd[:, C * 512 + c0 : (C + 1) * 512],
                                    start=True,
                                    stop=True,
                                )
                                if r >= 0 and "z" not in phases:
                                    nc.vector.tensor_add(
                                        pst[:, c0 : c0 + 128],
                                        pst[:, c0 : c0 + 128],
                                        maskt[:],
                                    )
                                es = esb.tile([128, 512], F32R, tag="es", bufs=es_bufs)
                                nc.scalar.activation(
                                    es[:, c0:512],
                                    pst[:, c0:512],
                                    mybir.ActivationFunctionType.Exp,
                                    scale=SCALE,
                                )
                                nc.tensor.matmul(
                                    po[:, c0:512],
                                    vv[:, j, :],
                                    es[:, c0:512],
                                    start=(j == 0),
                                    stop=(j == nj - 1),
                                )
                                if "y" not in phases:
                                    nc.tensor.matmul(
                                        pden[:, c0:512],
                                        ones128,
                                        es[:, c0:512],
                                        start=(j == 0),
                                        stop=(j == nj - 1),
                                    )
                            at = sb2.tile([128, 512], BF16, tag="at")
                            if "y" in phases:
                                nc.vector.tensor_copy(at[:], po[:])
                            else:
                                den = sb2.tile([1, 512], F32, tag="den")
                                nc.vector.tensor_copy(den[:], pden[:])
                                rec = sb2.tile([1, 512], F32R, tag="rec")
                                with nc.allow_low_precision(reason="f32r ~ f32"):
                                    nc.vector.reciprocal(rec[:], den[:])
                                pbc = ps.tile([128, 512], F32, tag="pbc", bufs=1)
                                nc.tensor.matmul(
                                    pbc[:], ones1, rec[:], start=True, stop=True
                                )
                                bcs = sb2.tile([128, 512], F32, tag="bcs")
                                nc.vector.tensor_copy(bcs[:], pbc[:])
                                nc.vector.tensor_mul(at[:], po[:], bcs[:])
                            nc.sync.dma_start(
                                ag_ins[b][
                                    hl * 128 : (hl + 1) * 128,
                                    C * 512 : (C + 1) * 512,
                                ],
                                at[:],
                            )

            # ================= phase D: AllGather + output projection ========
            if cfg.n_cores > 1 and "D" in phases:
                for b in range(B):
                    nc.gpsimd.collective_compute(
                        "AllGather",
                        mybir.AluOpType.bypass,
                        replica_groups=[list(range(cfg.n_cores))],
                        ins=[ag_ins[b][:].opt()],
                        outs=[ag_outs[b][:].opt()],
                    )
                ag_srcs = ag_outs
            elif cfg.hl_override:
                # single-core timing model: wo reads full-width dummies
                ag_srcs = [
                    dram.tile([D, S], BF16, name=f"ag_dummy{b}") for b in range(B)
                ]
            else:
                ag_srcs = ag_ins

            with (
                tc.tile_pool(name="wo_sb", bufs=1) as sb,
                tc.tile_pool(name="wo_ag", bufs=2) as agp,
                tc.tile_pool(name="wo_sb2", bufs=3) as sb2,
                tc.tile_pool(name="wo_ps", bufs=3, space="PSUM") as ps,
            ):
                wo = sb.tile([128, DT, DL], BF16)
                nc.sync.dma_start(wo[:], wo_ext[:])
                for n in range(TC if "D" in phases else 0):
                    t0 = n * 512
                    bb, sb0 = t0 // S, t0 % S
                    ag = agp.tile([128, DT, 512], BF16, tag="ag")
                    for a in range(DT):
                        nc.sync.dma_start(
                            ag[:, a, :],
                            ag_srcs[bb][a * 128 : (a + 1) * 128, sb0 : sb0 + 512],
                        )
                    for o in range(HL):
                        pw = ps.tile([128, 512], F32, tag="pw")
                        for a in range(DT):
                            nc.tensor.matmul(
                                pw[:],
                                wo[:, a, o * 128 : (o + 1) * 128],
                                ag[:, a, :],
                                start=(a == 0),
                                stop=(a == DT - 1),
                            )
                        ot = sb2.tile([128, 512], F32, tag="ot")
                        nc.vector.tensor_copy(ot[:], pw[:])
                        nc.sync.dma_start(
                            outt_ext[o * 128 : (o + 1) * 128, t0 : t0 + 512], ot[:]
                        )

    nc.compile()
    return nc


# ---------------------------------------------------------------------------
# host-side prep / assembly
# ---------------------------------------------------------------------------


def _rotary_perm(D, HD=128):
    """Within each head: even indices then odd indices."""
    per_head = np.concatenate([np.arange(0, HD, 2), np.arange(1, HD, 2)])
    return (np.arange(D // HD)[:, None] * HD + per_head[None, :]).reshape(-1)


def _wtile(w, DT):
    """[D, cols] -> [128, DT, cols] (d-in-tile major)."""
    D, cols = w.shape
    return np.ascontiguousarray(
        w.reshape(DT, 128, cols).transpose(1, 0, 2)
    )


def _check_structure(mask, idx, cfg):
    """Returns the [128, 128] transposed+prescaled diagonal mask tile, or
    None if the mask/indices don't fit the causal-structured fast path."""
    S = cfg.S
    if idx.shape != (S,) or not np.array_equal(idx, np.arange(S, dtype=idx.dtype)):
        return None
    m = np.asarray(mask, np.float32).reshape(S, S)
    kt = S // 128
    diag0 = m[0:128, 0:128]
    for j in range(kt):
        k0, k1 = j * 128, (j + 1) * 128
        # all diagonal tiles identical
        if j and not np.array_equal(m[k0:k1, k0:k1], diag0):
            return None
        # strictly below diagonal (q > k): must be exactly 0 (used unmasked)
        below = m[k1:, k0:k1]
        if below.size and np.abs(below).max() != 0.0:
            return None
        # strictly above diagonal (q < k): skipped, must be <= -1e8
        above = m[:k0, k0:k1]
        if above.size and above.max() > -1e8:
            return None
    return np.ascontiguousarray(diag0.T) * math.sqrt(128.0)


def _numpy_reference(x, freqs_cos, freqs_sin, mask, input_idexes, cache_k, cache_v,
                     wq, wk, wv, wo):
    B, S, D = x.shape
    HD = 128
    H = D // HD
    xq = (x @ wq.T).reshape(B, S, H, HD)
    xk = (x @ wk.T).reshape(B, S, H, HD)
    xv = (x @ wv.T).reshape(B, S, H, HD)

    def rot(t):
        tr = t.reshape(*t.shape[:-1], HD // 2, 2)
        te, to = tr[..., 0], tr[..., 1]
        c = freqs_cos[None, :, None, :]
        s = freqs_sin[None, :, None, :]
        oe = te * c - to * s
        oo = te * s + to * c
        return np.stack([oe, oo], axis=-1).reshape(t.shape)

    xq, xk = rot(xq), rot(xk)
    ck = np.array(cache_k, np.float32, copy=True)
    cv = np.array(cache_v, np.float32, copy=True)
    ck[:, input_idexes] = xk
    cv[:, input_idexes] = xv
    scores = np.einsum("bqhd,bkhd->bhqk", xq, ck) / math.sqrt(HD)
    scores = scores + mask
    scores = scores - scores.max(-1, keepdims=True)
    e = np.exp(scores)
    probs = e / e.sum(-1, keepdims=True)
    out = np.einsum("bhqk,bkhd->bqhd", probs, cv).reshape(B, S, H * HD)
    # reference: einsum('bso,do->bsd', out, wo) == out @ wo.T
    out = out @ wo.T
    return out.astype(np.float32), ck, cv


_CACHE = {}


PROD_PHASES = "MACD"


def _get_built(cfg_key, **cfg_kw):
    if cfg_key not in _CACHE:
        cfg = Cfg(**cfg_kw)
        nc = build(cfg, phases=PROD_PHASES, es_bufs=4, pst_bufs=4, pden_bufs=1)
        _CACHE[cfg_key] = (cfg, nc)
    return _CACHE[cfg_key]


def make_in_maps(cfg, x, freqs_cos, freqs_sin, mask_tiles, wq, wk, wv, wo):
    B, S, D, T, DT = cfg.B, cfg.S, cfg.D, cfg.T, cfg.DT
    DL = cfg.DL
    perm = _rotary_perm(D)
    wq_p, wk_p = wq[perm], wk[perm]
    xt_full = np.ascontiguousarray(
        x.reshape(T, D).T.reshape(DT, 128, T).transpose(1, 0, 2)
    ).astype(BF)
    cosT = np.ascontiguousarray(freqs_cos.T).astype(np.float32)
    sinT = np.ascontiguousarray(freqs_sin.T).astype(np.float32)
    ident = np.eye(128, dtype=np.float32)
    in_maps = []
    for c in range(cfg.n_cores):
        sl = slice(c * DL, (c + 1) * DL)
        wqk_c = np.concatenate([wq_p[sl].T, wk_p[sl].T], axis=1)  # [D, 2DL]
        in_maps.append(
            dict(
                xt=xt_full,
                wqk=_wtile(wqk_c, DT).astype(BF),
                wv=_wtile(wv[sl].T, DT).astype(BF),
                wo=_wtile(wo[sl].T, DT).astype(BF),
                cosT=cosT,
                sinT=sinT,
                maskt=np.asarray(mask_tiles, np.float32),
                ident=ident,
                ones=np.ones((128, 128), np.float32),
            )
        )
    return in_maps


def assemble(cfg, results, cache_k, cache_v, input_idexes):
    B, S = cfg.B, cfg.S
    outs, cks, cvs = [], [], []
    for r in results:
        outs.append(r["outt"].T.reshape(B, S, cfg.DL))
        cks.append(r["ck"])
        cvs.append(r["cv"])
    out = np.concatenate(outs, axis=2)
    k_new = np.concatenate(cks, axis=2)  # [B, S, H, 128]
    v_new = np.concatenate(cvs, axis=2)
    ck = np.array(cache_k, np.float32, copy=True)
    cv = np.array(cache_v, np.float32, copy=True)
    ck[:, np.asarray(input_idexes)] = k_new
    cv[:, np.asarray(input_idexes)] = v_new
    return out, ck, cv


def kernel(x, freqs_cos, freqs_sin, mask, input_idexes, cache_k, cache_v,
           wq, wk, wv, wo):
    x = np.asarray(x, np.float32)
    freqs_cos = np.asarray(freqs_cos, np.float32)
    freqs_sin = np.asarray(freqs_sin, np.float32)
    mask = np.asarray(mask, np.float32)
    idx = np.asarray(input_idexes)
    wq = np.asarray(wq, np.float32)
    wk = np.asarray(wk, np.float32)
    wv = np.asarray(wv, np.float32)
    wo = np.asarray(wo, np.float32)

    cfg = Cfg()
    mask_tiles = _check_structure(mask, idx, cfg)
    if (
        mask_tiles is None
        or x.shape != (cfg.B, cfg.S, cfg.D)
        or np.abs(np.asarray(cache_k)).max() != 0.0
        or np.abs(np.asarray(cache_v)).max() != 0.0
    ):
        out, ck, cv = _numpy_reference(
            x, freqs_cos, freqs_sin, mask, idx,
            np.asarray(cache_k), np.asarray(cache_v), wq, wk, wv, wo,
        )
        return (out, (ck, cv))

    cfg2, nc = _get_built("full")
    in_maps = make_in_maps(cfg2, x, freqs_cos, freqs_sin, mask_tiles, wq, wk, wv, wo)
    res = run_bass_kernel_spmd(nc, in_maps, core_ids=list(range(cfg2.n_cores)))
    out, ck, cv = assemble(cfg2, res.results, cache_k, cache_v, idx)
    return (out, (ck, cv))
